# revision 28
# baseline (speedup 1.0000x reference)
"""Trainium2 Bass kernel for the GNN message-passing autoencoder problem.

Strategy (8 NeuronCores, SPMD):
  - Nodes sharded 1024/core. Message passing is dense matmul against the
    per-core column shard of the *plain* adjacency transpose A^T[:, shard],
    stored fp8e4 and kept resident in SBUF for all 4 layers of each chain.
    GraphConv 'both' normalization is folded on-chip: D_src^-1/2 scales the
    producer's transposed output (per-partition scalar), D_dst^-1/2 scales
    the message during PSUM eviction (free-dim broadcast multiply).
  - A@h runs in fp8 DoubleRow perf mode (2 k-tiles per instruction). h is
    carried between layers as fp8 (quantization validated: final rel err
    ~2e-4). The W matmul, BN and PReLU stay bf16/f32 as before.
  - AllGathers move fp8 p-major payloads [128, SB*F]: producer writes its
    SBUF tile linearly; consumers read contiguous per-partition runs.
  - loss2 tail avoids the N x N product entirely:
      sum((h2n@h2n.T - M)^2) = ||G||_F^2 - 2<h2n, M@h2n> + ||M||_F^2
    with G = h2n^T h2n (512x512, AllReduced), M@h2n done fp8 DoubleRow
    against a streamed fp8 M^T shard, and ||M||^2 computed on host. h2n is
    scaled by 16 before fp8 quantization; the 256x factors divide out in
    the host combine step.
  - loss1 (sce) uses host-prenormalized attr rows; per-core shard partials
    are summed on host.
"""

import os
import sys

for _p in ("/opt/trn_rl_repo", "/opt/pypackages"):
    if _p not in sys.path:
        sys.path.append(_p)

import numpy as np
import ml_dtypes

import concourse.bass as bass
import concourse.mybir as mybir
import concourse.tile as tile
from concourse import bacc
from concourse.bass_utils import run_bass_kernel_spmd
from concourse.masks import make_identity

BF16 = mybir.dt.bfloat16
FP8 = mybir.dt.float8e4
F32 = mybir.dt.float32
AF = mybir.ActivationFunctionType
ALU = mybir.AluOpType
AX = mybir.AxisListType
DR = mybir.MatmulPerfMode.DoubleRow

N = 8192
F = 512
NCORES = 8
SH = N // NCORES          # 1024 nodes per core shard
NB = N // 128             # 64 node k-tiles
SB = SH // 128            # 8 node k-tiles per shard
FB = F // 128             # 4 feature blocks
KCH = 4                   # h k-tiles per DMA chunk (512 rows)
MCH = 2                   # M k-tiles per DMA chunk

# layer-instance parameter rows: enc0 enc1 dec1_0 dec1_1 dec2_0 dec2_1
LI = {1: [0, 1, 2, 3], 2: [0, 1, 4, 5]}


def _layer_front(nc, sb, ps, g, *, chain, lidx, li, h_chunk, h_order, a_sb,
                 load_a, a_dram, dd_b, ar_in, split_at=8):
    """Front half generator: A@h + W matmuls, bias+PReLU, BN stats, AR kick.

    Yields once after `split_at` chunks (injection point for the other
    chain's back-half so its transposes/AG-kick land mid-stream on the PE),
    then yields the zt tile (post first PReLU, pre BN) when complete.

    h_chunk(kk) -> [128, KCH, F] fp8 DRAM AP for k-chunk kk.
    h_order: chunk processing order (lo-AG-half chunks first for l>0).
    a_sb: resident [128, NB, SH] fp8 A^T shard tile (loaded here if load_a).
    dd_b: [128, SH] ddst broadcast tile or None (enc layers).
    """
    # ---- A @ h, fp8 DoubleRow, k streamed ----
    mps = [[ps.tile([128, 512], F32, tag="ps", name="ps") for _ in range(2)]
           for _ in range(FB)]
    nkc = NB // KCH
    for idx, kk in enumerate(h_order):
        if idx == split_at:
            yield None
        if load_a:
            nc.sync.dma_start(
                a_sb[:, kk * KCH:(kk + 1) * KCH, :],
                a_dram[:, kk * KCH * SH:(kk + 1) * KCH * SH]
                .rearrange("p (t d) -> p t d", t=KCH))
        hch = sb.tile([128, KCH, F], FP8, tag="hch", bufs=3, name="hch")
        nc.sync.dma_start(hch[:], h_chunk(kk))
        for kp in range(KCH // 2):
            t0 = kk * KCH + 2 * kp
            first = (idx == 0 and kp == 0)
            last = (idx == nkc - 1 and kp == KCH // 2 - 1)
            for m in range(FB):
                for n in range(2):
                    nc.tensor.matmul(
                        mps[m][n][:],
                        hch[:, 2 * kp:2 * kp + 2, m * 128:(m + 1) * 128],
                        a_sb[:, t0:t0 + 2, n * 512:(n + 1) * 512],
                        start=first, stop=last,
                        perf_mode=DR)

    # ---- evict mT (fold ddst for enc layers) ----
    mt = sb.tile([128, FB, SH], BF16, tag="mt", bufs=1, name="mt")
    for m in range(FB):
        for n in range(2):
            dst = mt[:, m, n * 512:(n + 1) * 512]
            if dd_b is not None:
                nc.vector.tensor_mul(dst, mps[m][n][:],
                                     dd_b[:, n * 512:(n + 1) * 512])
            else:
                if (m + n) % 2 == 0:
                    nc.vector.tensor_copy(dst, mps[m][n][:])
                else:
                    nc.scalar.copy(dst, mps[m][n][:])

    # ---- W matmul (bf16) ----
    wsb = sb.tile([128, FB, F], BF16, tag="w", bufs=1, name="w")
    nc.sync.dma_start(wsb[:], g["w_all"][li].rearrange("(t p) fo -> p t fo", p=128))
    zps = [[ps.tile([128, 512], F32, tag="ps", name="ps") for _ in range(2)]
           for _ in range(FB)]
    for m in range(FB):
        for n in range(2):
            for kb in range(FB):
                nc.tensor.matmul(
                    zps[m][n][:],
                    wsb[:, kb, m * 128:(m + 1) * 128],
                    mt[:, kb, n * 512:(n + 1) * 512],
                    start=(kb == 0), stop=(kb == FB - 1))

    # ---- bias + PReLU(ain) ----
    zt = sb.tile([128, FB, SH], BF16, tag="zt", bufs=2, name="zt")
    for m in range(FB):
        for n in range(2):
            nc.scalar.activation(
                zt[:, m, n * 512:(n + 1) * 512], zps[m][n][:], AF.Prelu,
                bias=g["b_sb"][:, li, m:m + 1], scale=1.0,
                alpha=g["al_sb"][:, 2 * li:2 * li + 1])

    # ---- BN stats + AllReduce ----
    stats = sb.tile([128, 8], F32, tag="stats", bufs=2, name="stats")
    for m in range(FB):
        nc.vector.reduce_sum(stats[:, 2 * m:2 * m + 1], zt[:, m, :], axis=AX.X)
        scr = sb.tile([128, SH], BF16, tag="scr", bufs=1, name="scr")
        nc.scalar.activation(scr[:], zt[:, m, :], AF.Square,
                             accum_out=stats[:, 2 * m + 1:2 * m + 2])
    nc.sync.dma_start(ar_in[:], stats[:])
    yield zt


def _layer_back(nc, sb, ps, g, *, chain, li, zt, ds_b, ag_in, ag_out,
                ar_in, ar_out):
    """Back half: AR collective, BN finalize+apply, transpose, AG kick."""
    nc.gpsimd.collective_compute(
        "AllReduce", ALU.add, replica_groups=[list(range(NCORES))],
        ins=[ar_in[:]], outs=[ar_out[:]])
    gstats = sb.tile([128, 8], F32, tag="gstats", bufs=2, name="gstats")
    nc.sync.dma_start(gstats[:], ar_out[:])

    # ---- finalize: s = g / sqrt(var+eps), t = bb - mean*s ----
    mean = sb.tile([128, FB], F32, tag="mean", bufs=2, name="mean")
    var = sb.tile([128, FB], F32, tag="var", bufs=2, name="var")
    sN = sb.tile([128, FB], F32, tag="sN", bufs=2, name="sN")
    tN = sb.tile([128, FB], F32, tag="tN", bufs=2, name="tN")
    m2 = sb.tile([128, FB], F32, tag="m2", bufs=2, name="m2")
    nc.scalar.mul(mean[:], gstats[:, 0:8:2], 1.0 / N)
    nc.scalar.mul(var[:], gstats[:, 1:8:2], 1.0 / N)
    nc.vector.tensor_mul(m2[:], mean[:], mean[:])
    nc.vector.tensor_sub(var[:], var[:], m2[:])
    nc.scalar.activation(sN[:], var[:], AF.Sqrt, bias=g["epsb"][:])
    nc.vector.reciprocal(sN[:], sN[:])
    nc.vector.tensor_mul(sN[:], sN[:], g["g_sb"][:, li, :])
    nc.vector.tensor_mul(m2[:], mean[:], sN[:])
    nc.vector.tensor_sub(tN[:], g["bb_sb"][:, li, :], m2[:])

    # ---- BN apply + PReLU(aout), in place ----
    for m in range(FB):
        nc.scalar.activation(
            zt[:, m, :], zt[:, m, :], AF.Prelu,
            bias=tN[:, m:m + 1], scale=sN[:, m:m + 1],
            alpha=g["al_sb"][:, 2 * li + 1:2 * li + 2])

    # ---- transpose to node-major fp8 (fold dsrc), split AllGather ----
    # Two half-gathers (t<4 / t>=4) so the consumer can start on the lo
    # half while the hi half is still in flight.
    if ag_in is not None:
        HB = SB // 2
        hnm = sb.tile([128, SB, F], FP8, tag="hnm", bufs=2, name="hnm")
        for half in range(2):
            for t in range(half * HB, (half + 1) * HB):
                for m in range(FB):
                    tp = ps.tile([128, 128], BF16, tag="ps", name="ps")
                    nc.tensor.transpose(tp[:], zt[:, m, t * 128:(t + 1) * 128],
                                        g["ident"][:])
                    dst = hnm[:, t, m * 128:(m + 1) * 128]
                    if ds_b is not None:
                        nc.vector.tensor_scalar_mul(dst, tp[:], ds_b[:, t:t + 1])
                    else:
                        nc.vector.tensor_copy(dst, tp[:])
            nc.sync.dma_start(
                ag_in[half].rearrange("p (t f) -> p t f", t=HB),
                hnm[:, half * HB:(half + 1) * HB, :])
            nc.gpsimd.collective_compute(
                "AllGather", ALU.bypass, replica_groups=[list(range(NCORES))],
                ins=[ag_in[half][:]], outs=[ag_out[half][:]])
    return zt


def build_nc():
    nc = bacc.Bacc("TRN2", target_bir_lowering=False, debug=False,
                   num_devices=NCORES)

    ins = {}
    def di(name, shape, dt):
        ins[name] = nc.dram_tensor(name, shape, dt, kind="ExternalInput")
        return ins[name]

    h1_0 = di("h1_0", [128, NB * F], FP8)    # (x * d1s) p-major fp8
    h2_0 = di("h2_0", [128, NB * F], FP8)    # (attr * d2s) p-major fp8
    a1 = di("a1", [128, NB * SH], FP8)       # plain A1^T shard, p-major
    a2 = di("a2", [128, NB * SH], FP8)
    msh = di("msh", [128, NB * SH], FP8)     # M^T column shard, p-major
    w_all = di("w_all", [6, F, F], BF16)
    b_all = di("b_all", [6, F], F32)
    g_all = di("g_all", [6, F], F32)
    bb_all = di("bb_all", [6, F], F32)
    al_all = di("al_all", [1, 12], F32)
    dd1 = di("dd1", [1, SH], BF16)           # d1d[shard]
    dd2 = di("dd2", [1, SH], BF16)
    ds1 = di("ds1", [128, SB], F32)          # d1s[shard] p-major
    ds2 = di("ds2", [128, SB], F32)
    attrn = di("attrn", [128, SB * F], BF16)  # prenormalized attr shard

    partials = nc.dram_tensor("partials", [4, 1], F32, kind="ExternalOutput")

    ag_in = {}
    ag_out = {}
    ar_in = {}
    ar_out = {}
    for c in (1, 2):
        for l in range(4):
            ar_in[(c, l)] = nc.dram_tensor(f"ar_in_{c}_{l}", [128, 8], F32)
            ar_out[(c, l)] = nc.dram_tensor(f"ar_out_{c}_{l}", [128, 8], F32,
                                            addr_space="Shared")
            if l < 3:
                ag_in[(c, l)] = [
                    nc.dram_tensor(f"ag_in_{c}_{l}_{h}", [128, SB // 2 * F],
                                   FP8) for h in range(2)]
                ag_out[(c, l)] = [
                    nc.dram_tensor(f"ag_out_{c}_{l}_{h}",
                                   [NCORES * 128, SB // 2 * F], FP8,
                                   addr_space="Shared") for h in range(2)]
    ag2_in = [nc.dram_tensor(f"ag2_in_{h}", [128, SB // 2 * F], FP8)
              for h in range(2)]
    ag2_out = [nc.dram_tensor(f"ag2_out_{h}", [NCORES * 128, SB // 2 * F], FP8,
                              addr_space="Shared") for h in range(2)]
    war_in = nc.dram_tensor("war_in", [128, 4], F32)
    war_out = nc.dram_tensor("war_out", [128, 4], F32, addr_space="Shared")
    wag_in = nc.dram_tensor("wag_in", [128, 4], F32)
    wag_out = nc.dram_tensor("wag_out", [NCORES * 128, 4], F32,
                             addr_space="Shared")
    gar_in = nc.dram_tensor("gar_in", [128, FB * 512], BF16)
    gar_out = nc.dram_tensor("gar_out", [128, FB * 512], BF16,
                             addr_space="Shared")

    dbg = {}
    if os.environ.get("BASSK_DEBUG"):
        for c in (1, 2):
            for l in range(3):
                dbg[(c, l)] = nc.dram_tensor(f"dbg_h_{c}_{l}", [128, SB * F],
                                             FP8, kind="ExternalOutput")
        dbg["h2n16"] = nc.dram_tensor("dbg_h2n16", [128, SB * F], FP8,
                                      kind="ExternalOutput")

    def h0_chunk(h0):
        def f(kk):
            return h0[:, kk * KCH * F:(kk + 1) * KCH * F] \
                .rearrange("p (t f) -> p t f", t=KCH)
        return f

    def ag_chunk(ago_halves):
        # chunk kk = c8*2 + j: exactly one AG half-block (KCH == SB//2)
        def f(kk):
            c8, j = divmod(kk, 2)
            return ago_halves[j][c8 * 128:(c8 + 1) * 128, :] \
                .rearrange("p (t f) -> p t f", t=KCH)
        return f

    # lo-half chunks (j=0) first so the A@h can start as soon as the lo
    # AllGather lands; hi chunks follow while the hi gather is in flight.
    AG_ORDER = [c8 * 2 for c8 in range(NCORES)] + \
               [c8 * 2 + 1 for c8 in range(NCORES)]

    with tile.TileContext(nc) as tc:
        with (
            tc.tile_pool(name="sb", bufs=2) as sb,
            tc.tile_pool(name="ps", bufs=8, space="PSUM") as ps,
        ):
            # ---- constants / params ----
            g = {"w_all": w_all}
            ident = sb.tile([128, 128], BF16, tag="ident", bufs=1, name="ident")
            make_identity(nc, ident[:])
            g["ident"] = ident
            for nm, src in (("b_sb", b_all), ("g_sb", g_all), ("bb_sb", bb_all)):
                t = sb.tile([128, 6, FB], F32, tag=nm, bufs=1)
                nc.sync.dma_start(t[:], src.rearrange("l (m p) -> p l m", p=128))
                g[nm] = t
            al1 = sb.tile([1, 12], F32, tag="al1", bufs=1, name="al1")
            nc.sync.dma_start(al1[:], al_all[:])
            al_sb = sb.tile([128, 12], F32, tag="al_sb", bufs=1, name="al_sb")
            nc.gpsimd.partition_broadcast(al_sb[:], al1[:])
            g["al_sb"] = al_sb
            epsb = sb.tile([128, 1], F32, tag="epsb", bufs=1, name="epsb")
            nc.vector.memset(epsb[:], 1e-5)
            g["epsb"] = epsb

            ddb = {}
            dsb = {}
            for c, (ddi, dsi) in ((1, (dd1, ds1)), (2, (dd2, ds2))):
                d1 = sb.tile([1, SH], BF16, tag=f"dd1_{c}", bufs=1)
                nc.sync.dma_start(d1[:], ddi[:])
                db = sb.tile([128, SH], BF16, tag=f"ddb_{c}", bufs=1)
                nc.gpsimd.partition_broadcast(db[:], d1[:])
                ddb[c] = db
                dst = sb.tile([128, SB], F32, tag=f"dsb_{c}", bufs=1)
                nc.sync.dma_start(dst[:], dsi[:])
                dsb[c] = dst

            a_sb = {
                1: sb.tile([128, NB, SH], FP8, tag="a1", bufs=1, name="a1"),
                2: sb.tile([128, NB, SH], FP8, tag="a2", bufs=1, name="a2"),
            }
            a_dram = {1: a1, 2: a2}

            # ---- warm up the collective paths during the A-load phase ----
            wz = sb.tile([128, 4], F32, tag="wz", bufs=1, name="wz")
            nc.vector.memset(wz[:], 0.0)
            nc.sync.dma_start(war_in[:], wz[:])
            nc.gpsimd.collective_compute(
                "AllReduce", ALU.add, replica_groups=[list(range(NCORES))],
                ins=[war_in[:]], outs=[war_out[:]])
            nc.sync.dma_start(wag_in[:], wz[:])
            nc.gpsimd.collective_compute(
                "AllGather", ALU.bypass, replica_groups=[list(range(NCORES))],
                ins=[wag_in[:]], outs=[wag_out[:]])

            # ---- 2 chains x 4 layers, software-pipelined: each chain's
            # back-half (BN apply + transposes + AG kick) is injected into
            # the middle of the other chain's matmul stream, and back(c2)
            # is delayed into the next pair so its AG hides under the next
            # c1 front. Steady-state PE queue per pair:
            #   [a1, Y(prev c2 back), a2, b1, X(c1 back), b2] ----
            hT_final = {}
            hcur = {1: h0_chunk(h1_0), 2: h0_chunk(h2_0)}
            orders = {1: list(range(NB // KCH)), 2: list(range(NB // KCH))}

            def emit_back(c, l, zt):
                last = (l == 3)
                out = _layer_back(
                    nc, sb, ps, g, chain=c, li=LI[c][l], zt=zt,
                    ds_b=dsb[c] if l == 0 else None,
                    ag_in=None if last else ag_in[(c, l)],
                    ag_out=None if last else ag_out[(c, l)],
                    ar_in=ar_in[(c, l)], ar_out=ar_out[(c, l)])
                if not last:
                    if (c, l) in dbg:
                        HF = SB // 2 * F
                        nc.sync.dma_start(dbg[(c, l)][:, :HF],
                                          ag_in[(c, l)][0][:])
                        nc.sync.dma_start(dbg[(c, l)][:, HF:],
                                          ag_in[(c, l)][1][:])
                    hcur[c] = ag_chunk(ag_out[(c, l)])
                    orders[c] = AG_ORDER
                else:
                    hT_final[c] = out

            def front_gen(c, l):
                return _layer_front(
                    nc, sb, ps, g, chain=c, lidx=l, li=LI[c][l],
                    h_chunk=hcur[c], h_order=orders[c], a_sb=a_sb[c],
                    load_a=(l == 0), a_dram=a_dram[c],
                    dd_b=ddb[c] if l < 2 else None,
                    ar_in=ar_in[(c, l)])

            y_pending = None  # (l, zt) for the delayed back(c2)
            for l in range(4):
                g1 = front_gen(1, l)
                next(g1)                       # a1: c1 lo-half chunks
                if y_pending is not None:
                    emit_back(2, *y_pending)   # Y: prev pair's c2 back
                zt1 = next(g1)                 # a2: c1 front complete
                g2 = front_gen(2, l)
                next(g2)                       # b1: c2 lo-half chunks
                emit_back(1, l, zt1)           # X: c1 back (AG mid-stream)
                zt2 = next(g2)                 # b2: c2 front complete
                y_pending = (l, zt2)

            _stop = os.environ.get("BASSK_STOP", "full")
            _lvl = {"layers": 0, "tail1": 1, "tail2pre": 2, "full": 3}[_stop]

            cacc = sb.tile([128, SB], F32, tag="cacc", bufs=1, name="cacc")
            gq = sb.tile([128, 2], F32, tag="gq", bufs=1, name="gq")
            l1p = sb.tile([128, 1], F32, tag="l1p", bufs=1, name="l1p")
            nc.vector.memset(cacc[:], 0.0)
            nc.vector.memset(gq[:], 0.0)
            nc.vector.memset(l1p[:], 0.0)

            # ---- tail1: loss1 partial over shard; its PE transposes also
            # cover the final c2 AllReduce latency ----
            if _lvl >= 1:
                zt1 = hT_final[1]
                dot = sb.tile([128, SB], F32, tag="dot", bufs=1, name="dot")
                n1 = sb.tile([128, SB], F32, tag="n1", bufs=1, name="n1")
                for t in range(SB):
                    h1t = sb.tile([128, F], BF16, tag="h1t", bufs=2, name="h1t")
                    for m in range(FB):
                        tp = ps.tile([128, 128], BF16, tag="ps", name="ps")
                        nc.tensor.transpose(tp[:], zt1[:, m, t * 128:(t + 1) * 128],
                                            g["ident"][:])
                        nc.vector.tensor_copy(h1t[:, m * 128:(m + 1) * 128], tp[:])
                    at = sb.tile([128, F], BF16, tag="scr2", bufs=3, name="at")
                    nc.sync.dma_start(at[:], attrn[:, t * F:(t + 1) * F])
                    dscr = sb.tile([128, F], BF16, tag="scr2", bufs=3, name="dscr")
                    nc.vector.tensor_mul(dscr[:], h1t[:], at[:])
                    nc.vector.reduce_sum(dot[:, t:t + 1], dscr[:], axis=AX.X)
                    sscr = sb.tile([128, F], BF16, tag="scr2", bufs=3, name="sscr")
                    nc.scalar.activation(sscr[:], h1t[:], AF.Square,
                                         accum_out=n1[:, t:t + 1])
                p1 = sb.tile([128, SB], F32, tag="p1", bufs=1, name="p1")
                nc.scalar.activation(p1[:], n1[:], AF.Sqrt)
                nc.vector.reciprocal(p1[:], p1[:])
                nc.vector.tensor_mul(dot[:], dot[:], p1[:])
                u = sb.tile([128, SB], F32, tag="u", bufs=1, name="u")
                nc.scalar.activation(u[:], dot[:], AF.Copy, scale=-1.0, bias=1.0)
                u2 = sb.tile([128, SB], F32, tag="u2", bufs=1, name="u2")
                nc.vector.tensor_mul(u2[:], u[:], u[:])
                nc.vector.tensor_mul(u2[:], u2[:], u[:])
                nc.vector.reduce_sum(l1p[:], u2[:], axis=AX.X)

            # the delayed final back(c2, 3): BN for the h2 chain tail
            emit_back(2, *y_pending)

            # ---- tail2 pre: h2 -> h2n16 fp8, AG2 kick, G + AllReduce ----
            if _lvl >= 2:
                zt2 = hT_final[2]
                h2nm = sb.tile([128, SB, F], BF16, tag="mt", bufs=1, name="h2nm")
                for t in range(SB):
                    for m in range(FB):
                        tp = ps.tile([128, 128], BF16, tag="ps", name="ps")
                        nc.tensor.transpose(tp[:], zt2[:, m, t * 128:(t + 1) * 128],
                                            g["ident"][:])
                        nc.vector.tensor_copy(h2nm[:, t, m * 128:(m + 1) * 128],
                                              tp[:])
                nrm = sb.tile([128, SB], F32, tag="nrm", bufs=1, name="nrm")
                for t in range(SB):
                    scr2 = sb.tile([128, F], BF16, tag="scr2", bufs=3, name="scr2")
                    nc.scalar.activation(scr2[:], h2nm[:, t, :], AF.Square,
                                         accum_out=nrm[:, t:t + 1])
                nc.scalar.activation(nrm[:], nrm[:], AF.Sqrt)
                nc.vector.tensor_scalar_max(nrm[:], nrm[:], 1e-12)
                nc.vector.reciprocal(nrm[:], nrm[:])
                nc.scalar.mul(nrm[:], nrm[:], 16.0)
                h2n16 = sb.tile([128, SB, F], FP8, tag="h2n16", bufs=1,
                                name="h2n16")
                HB = SB // 2
                for half in range(2):
                    for t in range(half * HB, (half + 1) * HB):
                        nc.vector.tensor_scalar_mul(h2n16[:, t, :],
                                                    h2nm[:, t, :],
                                                    nrm[:, t:t + 1])
                    nc.sync.dma_start(
                        ag2_in[half].rearrange("p (t f) -> p t f", t=HB),
                        h2n16[:, half * HB:(half + 1) * HB, :])
                    nc.gpsimd.collective_compute(
                        "AllGather", ALU.bypass,
                        replica_groups=[list(range(NCORES))],
                        ins=[ag2_in[half][:]], outs=[ag2_out[half][:]])
                if "h2n16" in dbg:
                    HF = SB // 2 * F
                    nc.sync.dma_start(dbg["h2n16"][:, :HF], ag2_in[0][:])
                    nc.sync.dma_start(dbg["h2n16"][:, HF:], ag2_in[1][:])

                # G256 partial over shard
                gps = [ps.tile([128, 512], F32, tag="ps", name="ps")
                       for _ in range(FB)]
                for m in range(FB):
                    for kp in range(SB // 2):
                        nc.tensor.matmul(
                            gps[m][:],
                            h2n16[:, 2 * kp:2 * kp + 2, m * 128:(m + 1) * 128],
                            h2n16[:, 2 * kp:2 * kp + 2, :],
                            start=(kp == 0), stop=(kp == SB // 2 - 1),
                            perf_mode=DR)
                gsb = sb.tile([128, FB, 512], BF16, tag="gsb", bufs=1, name="gsb")
                for m in range(FB):
                    if m % 2 == 0:
                        nc.vector.tensor_copy(gsb[:, m, :], gps[m][:])
                    else:
                        nc.scalar.copy(gsb[:, m, :], gps[m][:])
                nc.sync.dma_start(gar_in.rearrange("p (t f) -> p t f", t=FB),
                                  gsb[:])
                nc.gpsimd.collective_compute(
                    "AllReduce", ALU.add, replica_groups=[list(range(NCORES))],
                    ins=[gar_in[:]], outs=[gar_out[:]])

            # ---- tail2: ||G||^2, M @ h2n, cross term ----
            if _lvl >= 2:
                gsum = sb.tile([128, FB * 512], BF16, tag="gsb", bufs=1,
                               name="gsum")
                nc.sync.dma_start(gsum[:], gar_out[:])
                for hh in range(2):
                    gscr = sb.tile([128, SH], BF16, tag="scr", bufs=1,
                                   name="gscr")
                    nc.scalar.activation(gscr[:], gsum[:, hh * SH:(hh + 1) * SH],
                                         AF.Square, accum_out=gq[:, hh:hh + 1])

            if _lvl >= 3:
                yps = [ps.tile([128, 512], F32, tag="ps", name="ps")
                       for _ in range(SB)]
                nmc = NB // MCH
                cpb = SB // MCH  # M chunks per core block
                hpb = cpb // 2   # M chunks per AG half-block
                m_order = [c8 * cpb + j for half in range(2)
                           for c8 in range(NCORES)
                           for j in range(half * hpb, (half + 1) * hpb)]
                for idx, kk in enumerate(m_order):
                    mch = sb.tile([128, MCH, SH], FP8, tag="mch", bufs=2,
                                  name="mch")
                    nc.sync.dma_start(
                        mch[:], msh[:, kk * MCH * SH:(kk + 1) * MCH * SH]
                        .rearrange("p (t d) -> p t d", t=MCH))
                    h2ch = sb.tile([128, MCH, F], FP8, tag="hch", bufs=3,
                                   name="h2ch")
                    c8, j = divmod(kk, cpb)
                    half, jh = divmod(j, hpb)
                    nc.sync.dma_start(
                        h2ch[:], ag2_out[half][c8 * 128:(c8 + 1) * 128,
                                             jh * MCH * F:(jh + 1) * MCH * F]
                        .rearrange("p (t f) -> p t f", t=MCH))
                    for mb in range(SB):
                        nc.tensor.matmul(
                            yps[mb][:],
                            mch[:, 0:2, mb * 128:(mb + 1) * 128],
                            h2ch[:, 0:2, :],
                            start=(idx == 0), stop=(idx == nmc - 1),
                            perf_mode=DR)
                # cross term: cacc[:, mb] = nrm16 .* sum_f(y .* h2nm)
                # (uses bf16 h2nm * nrm16 instead of the fp8-rounded h2n16;
                #  the consistency error in the expansion is ~1e-7 relative)
                for mb in range(SB):
                    cscr = sb.tile([128, F], BF16, tag="scr2", bufs=3,
                                   name="cscr")
                    nc.vector.tensor_mul(cscr[:], yps[mb][:], h2nm[:, mb, :])
                    nc.vector.reduce_sum(cacc[:, mb:mb + 1], cscr[:], axis=AX.X)
                nc.vector.tensor_mul(cacc[:], cacc[:], nrm[:])

            # ---- combine partials ----
            pl = sb.tile([128, 4], F32, tag="pl", bufs=1, name="pl")
            nc.vector.memset(pl[:], 0.0)
            nc.vector.tensor_copy(pl[:, 0:1], l1p[:])
            nc.vector.reduce_sum(pl[:, 1:2], cacc[:], axis=AX.X)
            nc.vector.reduce_sum(pl[:, 2:3], gq[:], axis=AX.X)
            ones = sb.tile([128, 1], F32, tag="ones", bufs=1, name="ones")
            nc.vector.memset(ones[:], 1.0)
            pp = ps.tile([4, 1], F32, tag="ps", name="pp")
            nc.tensor.matmul(pp[:], pl[:], ones[:], start=True, stop=True)
            out_sb = sb.tile([4, 1], F32, tag="out_sb", bufs=1, name="out_sb")
            nc.scalar.copy(out_sb[:], pp[:])
            nc.sync.dma_start(partials[:], out_sb[:])

    nc.compile()
    return nc


_NC_CACHE = None


def _get_nc():
    global _NC_CACHE
    if _NC_CACHE is None:
        _NC_CACHE = build_nc()
    return _NC_CACHE


def _dinv(idx):
    deg = np.bincount(idx, minlength=N).astype(np.float32)
    return 1.0 / np.sqrt(np.clip(deg, 1.0, None))


def _adj_t(src, dst):
    """A^T[s, d] = multiplicity of edge s->d, float32 [N, N]."""
    flat = src.astype(np.int64) * N + dst.astype(np.int64)
    return np.bincount(flat, minlength=N * N).astype(np.float32).reshape(N, N)


def _pmaj(x, blk):
    """[NB*128, blk] -> [128, NB*blk] p-major (k-tile order preserved)."""
    nt = x.shape[0] // 128
    return np.ascontiguousarray(
        x.reshape(nt, 128, blk).transpose(1, 0, 2).reshape(128, nt * blk))


def host_prep(inputs):
    bf16 = ml_dtypes.bfloat16
    fp8 = ml_dtypes.float8_e4m3
    attr = np.asarray(inputs["attr"], np.float32)
    matrix = np.asarray(inputs["matrix"], np.float32)
    mask1 = np.asarray(inputs["enc_mask_token1"], np.float32)
    src = np.asarray(inputs["src"]); dst = np.asarray(inputs["dst"])
    src2 = np.asarray(inputs["src2"]); dst2 = np.asarray(inputs["dst2"])
    tok = np.asarray(inputs["token_nodes"])
    noi = np.asarray(inputs["noise_nodes"])
    nsrc = np.asarray(inputs["noise_src"])

    x = attr.copy()
    x[tok] = 0.0
    x[noi] = attr[nsrc]
    np.add.at(x, tok, mask1[0])

    d1s, d1d = _dinv(src), _dinv(dst)
    d2s, d2d = _dinv(src2), _dinv(dst2)

    a1t = _adj_t(src, dst).astype(fp8)
    a2t = _adj_t(src2, dst2).astype(fp8)

    h1_0 = _pmaj((x * d1s[:, None]).astype(fp8), F)
    h2_0 = _pmaj((attr * d2s[:, None]).astype(fp8), F)

    attr_n = (attr / np.maximum(
        np.linalg.norm(attr, axis=1, keepdims=True), 1e-12)).astype(bf16)

    w_all = np.stack([
        np.asarray(inputs["enc_W"][0]), np.asarray(inputs["enc_W"][1]),
        np.asarray(inputs["dec1_W"][0]), np.asarray(inputs["dec1_W"][1]),
        np.asarray(inputs["dec2_W"][0]), np.asarray(inputs["dec2_W"][1]),
    ]).astype(bf16)

    def stack6(key):
        return np.stack([
            np.asarray(inputs[f"enc_{key}"][0]), np.asarray(inputs[f"enc_{key}"][1]),
            np.asarray(inputs[f"dec1_{key}"][0]), np.asarray(inputs[f"dec1_{key}"][1]),
            np.asarray(inputs[f"dec2_{key}"][0]), np.asarray(inputs[f"dec2_{key}"][1]),
        ]).astype(np.float32)

    b_all, g_all, bb_all = stack6("b"), stack6("g"), stack6("bb")
    al = np.zeros((1, 12), np.float32)
    for i, (sa, so) in enumerate((("enc", 0), ("enc", 1), ("dec1", 0),
                                  ("dec1", 1), ("dec2", 0), ("dec2", 1))):
        al[0, 2 * i] = np.asarray(inputs[f"{sa}_ain"])[so]
        al[0, 2 * i + 1] = np.asarray(inputs[f"{sa}_aout"])[so]

    aux = {"msq": float(np.dot(matrix.reshape(-1).astype(np.float64),
                               matrix.reshape(-1).astype(np.float64)))}

    in_maps = []
    for c in range(NCORES):
        sl = slice(c * SH, (c + 1) * SH)
        in_maps.append({
            "h1_0": h1_0, "h2_0": h2_0,
            "a1": _pmaj(np.ascontiguousarray(a1t[:, sl]), SH),
            "a2": _pmaj(np.ascontiguousarray(a2t[:, sl]), SH),
            "msh": _pmaj(np.ascontiguousarray(matrix[sl, :].T).astype(fp8), SH),
            "w_all": w_all, "b_all": b_all, "g_all": g_all, "bb_all": bb_all,
            "al_all": al,
            "dd1": np.ascontiguousarray(d1d[sl])[None, :].astype(bf16),
            "dd2": np.ascontiguousarray(d2d[sl])[None, :].astype(bf16),
            "ds1": np.ascontiguousarray(d1s[sl].reshape(SB, 128).T),
            "ds2": np.ascontiguousarray(d2s[sl].reshape(SB, 128).T),
            "attrn": _pmaj(np.ascontiguousarray(attr_n[sl]), F),
        })
    return in_maps, aux


def combine(results, aux):
    l1 = sum(float(r["partials"][0, 0]) for r in results)
    cross256 = sum(float(r["partials"][1, 0]) for r in results)
    gsq = float(results[0]["partials"][2, 0])
    loss1 = l1 / N
    loss2 = (gsq / 65536.0 - 2.0 * cross256 / 256.0 + aux["msq"]) / (float(N) * N)
    loss = 0.5 * loss1 + 0.5 * loss2
    return np.asarray(loss, dtype=np.float32)


def run(inputs, trace=False, trace_kwargs=None):
    nc = _get_nc()
    in_maps, aux = host_prep(inputs)
    res = run_bass_kernel_spmd(nc, in_maps, core_ids=list(range(NCORES)),
                               trace=trace, **(trace_kwargs or {}))
    return combine(res.results, aux), res


def kernel(**inputs) -> np.ndarray:
    out, _ = run(inputs, trace=False)
    return out
